# revision 1
# baseline (speedup 1.0000x reference)
"""Grouped GEMM (MoE routing) Trainium2 kernel.

Expert-parallel across 8 NeuronCores with size-sorted slot assignment:
experts are sorted by token count and slot s on every core holds the
experts of size-rank [8s, 8s+8), so one SPMD program with per-slot
capacities cap_s = roundup32(max count in rank group) serves all cores
with ~40% less padding than a fixed CAP=256.

Per slot: out[cap_s, DOUT] = x[cap_s, DIN] @ w[DIN, DOUT] on the PE in
bfloat16 (~2.6e-3 rel err), lhsT = host-transposed token
tiles, rhs = weight K-slabs [128, DOUT] streamed through SBUF,
accumulating over 20 K-chunks in PSUM ([128, 416] tiles, 4 DOUT chunks).
"""
import ml_dtypes
import numpy as np

import concourse.bass as bass
import concourse.mybir as mybir
import concourse.tile as tile
from concourse import bacc
from concourse.bass_utils import run_bass_kernel_spmd

G, T, DIN, DOUT = 64, 8192, 2560, 1664
NCORES = 8
EPC = G // NCORES   # expert slots per core
KC = DIN // 128     # 20 contraction chunks
NT = 4              # DOUT chunks
NW = DOUT // NT     # 416 (<=512 fp32 PSUM bank, >=256 for full-rate f32r)

_cache = {}


def _build(caps):
    offs = np.concatenate([[0], np.cumsum(caps)]).astype(int)
    sumcap = int(offs[-1])
    nc = bacc.Bacc(trn_type="TRN2", debug=False)
    bf16 = mybir.dt.bfloat16
    xt = nc.dram_tensor("xt", [DIN, sumcap], bf16, kind="ExternalInput").ap()
    w = nc.dram_tensor("w", [EPC, DIN, DOUT], bf16, kind="ExternalInput").ap()
    out = nc.dram_tensor(
        "out", [sumcap, DOUT], mybir.dt.float32, kind="ExternalOutput"
    ).ap()
    with tile.TileContext(nc) as tc:
        with (
            tc.tile_pool(name="xtp", bufs=3) as xt_pool,
            tc.tile_pool(name="wp", bufs=12) as w_pool,
            tc.tile_pool(name="op", bufs=4) as o_pool,
            tc.tile_pool(name="ps", bufs=1, space="PSUM") as ps_pool,
        ):
            for s in range(EPC):
                cap = int(caps[s])
                off = int(offs[s])
                mts = (cap + 127) // 128  # m-tiles in this slot
                xt_sb = xt_pool.tile([128, KC * cap], bf16, tag="xt", name=f"xt{s}")
                nc.gpsimd.dma_start(
                    xt_sb[:].rearrange("p (c t) -> p c t", c=KC),
                    xt[:, off:off + cap].rearrange("(c p) t -> p c t", p=128),
                )
                psums = {}
                for m in range(mts):
                    for n in range(NT):
                        psums[m, n] = ps_pool.tile(
                            [128, NW], mybir.dt.float32, tag=f"ps{m}{n}",
                            name=f"psum_{s}_{m}_{n}",
                        )
                for k in range(KC):
                    w_sb = w_pool.tile([128, DOUT], bf16, tag="w", name=f"w{s}_{k}")
                    nc.sync.dma_start(w_sb[:], w[s, k * 128:(k + 1) * 128, :])
                    for m in range(mts):
                        msz = min(128, cap - m * 128)
                        for n in range(NT):
                            nc.tensor.matmul(
                                psums[m, n][:msz],
                                xt_sb[:, k * cap + m * 128: k * cap + m * 128 + msz],
                                w_sb[:, n * NW:(n + 1) * NW],
                                start=(k == 0),
                                stop=(k == KC - 1),
                            )
                for m in range(mts):
                    msz = min(128, cap - m * 128)
                    o_sb = o_pool.tile([128, DOUT], mybir.dt.float32, tag="o",
                                       name=f"o_{s}_{m}")
                    for n in range(NT):
                        nc.vector.tensor_copy(
                            o_sb[:msz, n * NW:(n + 1) * NW], psums[m, n][:msz]
                        )
                    nc.scalar.dma_start(
                        out[off + m * 128: off + m * 128 + msz, :], o_sb[:msz]
                    )
    nc.compile()
    return nc


def _run(inputs, trace=False):
    x = np.asarray(inputs["input"], dtype=np.float32)
    w = np.ascontiguousarray(np.asarray(inputs["weight"], dtype=np.float32))
    counts = np.asarray(inputs["tokens_per_expert"], dtype=np.int64)
    starts = np.concatenate([[0], np.cumsum(counts)[:-1]])

    order = np.argsort(-counts, kind="stable")  # experts by size rank
    # slot s, core c -> expert order[s*NCORES + c]; capacity = rank-group max
    caps = tuple(
        int(np.ceil(max(1, counts[order[s * NCORES:(s + 1) * NCORES]].max()) / 32) * 32)
        for s in range(EPC)
    )
    offs = np.concatenate([[0], np.cumsum(caps)]).astype(int)
    sumcap = int(offs[-1])

    if caps not in _cache:
        _cache[caps] = _build(caps)
    nc = _cache[caps]

    in_maps = []
    for c in range(NCORES):
        xt_pack = np.zeros((DIN, sumcap), dtype=ml_dtypes.bfloat16)
        w_pack = np.empty((EPC, DIN, DOUT), dtype=ml_dtypes.bfloat16)
        for s in range(EPC):
            g = int(order[s * NCORES + c])
            cnt = int(counts[g])
            if cnt:
                xt_pack[:, offs[s]:offs[s] + cnt] = x[starts[g]:starts[g] + cnt].T
            w_pack[s] = w[g]
        in_maps.append({"xt": xt_pack, "w": w_pack})

    kw = {"trace_cores": list(range(NCORES))} if trace else {}
    res = run_bass_kernel_spmd(nc, in_maps, core_ids=list(range(NCORES)),
                               trace=trace, **kw)

    out = np.empty((T, DOUT), dtype=np.float32)
    for c in range(NCORES):
        for s in range(EPC):
            g = int(order[s * NCORES + c])
            cnt = int(counts[g])
            if cnt:
                out[starts[g]:starts[g] + cnt] = \
                    res.results[c]["out"][offs[s]:offs[s] + cnt]
    return out, res


def kernel(**inputs) -> np.ndarray:
    return _run(inputs)[0]



# revision 2
# speedup vs baseline: 1.3303x; 1.3303x over previous
"""Grouped GEMM (MoE routing) Trainium2 kernel.

Expert-parallel across 8 NeuronCores with size-sorted slot assignment:
experts are sorted by token count and slot s on every core holds the
experts of size-rank [8s, 8s+8), so one SPMD program with per-slot
capacities cap_s = roundup4(max count in rank group) serves all cores.

Weights are streamed as fp8 E3M4 (scaled by 64, the 1/64 folded into
the bf16 x pack) to halve the dominant HBM traffic; the PE runs
bf16 (stationary x tile) x fp8e3 (moving w slab) matmuls at 1
col/cycle, accumulating over 20 K-chunks in PSUM ([128, 416] f32
tiles, 4 DOUT chunks), then evacuates to bf16 output (~1.2e-2 rel
err from the e3m4 weight quantization).
"""
import ml_dtypes
import numpy as np

import concourse.bass as bass
import concourse.mybir as mybir
import concourse.tile as tile
from concourse import bacc
from concourse.bass_utils import run_bass_kernel_spmd

G, T, DIN, DOUT = 64, 8192, 2560, 1664
NCORES = 8
EPC = G // NCORES   # expert slots per core
KC = DIN // 128     # 20 contraction chunks
NT = 4              # DOUT chunks
NW = DOUT // NT     # 416 (<=512 fp32 PSUM bank)
WSCALE = 64.0       # fp8 e3m4 weight scale (folded back via x/WSCALE)

_cache = {}


def _build(caps):
    offs = np.concatenate([[0], np.cumsum(caps)]).astype(int)
    sumcap = int(offs[-1])
    nc = bacc.Bacc(trn_type="TRN2", debug=False)
    bf16 = mybir.dt.bfloat16
    f8 = mybir.dt.float8e3
    xt = nc.dram_tensor("xt", [DIN, sumcap], bf16, kind="ExternalInput").ap()
    w = nc.dram_tensor("w", [EPC, DIN, DOUT], f8, kind="ExternalInput").ap()
    out = nc.dram_tensor(
        "out", [sumcap, DOUT], bf16, kind="ExternalOutput"
    ).ap()
    with tile.TileContext(nc) as tc:
        with (
            tc.tile_pool(name="xtp", bufs=3) as xt_pool,
            tc.tile_pool(name="wp", bufs=12) as w_pool,
            tc.tile_pool(name="op", bufs=4) as o_pool,
            tc.tile_pool(name="ps", bufs=1, space="PSUM") as ps_pool,
        ):
            for s in range(EPC):
                cap = int(caps[s])
                off = int(offs[s])
                mts = (cap + 127) // 128  # m-tiles in this slot
                xt_sb = xt_pool.tile([128, KC * cap], bf16, tag="xt", name=f"xt{s}")
                nc.gpsimd.dma_start(
                    xt_sb[:].rearrange("p (c t) -> p c t", c=KC),
                    xt[:, off:off + cap].rearrange("(c p) t -> p c t", p=128),
                )
                psums = {}
                for m in range(mts):
                    for n in range(NT):
                        psums[m, n] = ps_pool.tile(
                            [128, NW], mybir.dt.float32, tag=f"ps{m}{n}",
                            name=f"psum_{s}_{m}_{n}",
                        )
                for k in range(KC):
                    w_sb = w_pool.tile([128, DOUT], f8, tag="w", name=f"w{s}_{k}")
                    nc.sync.dma_start(w_sb[:], w[s, k * 128:(k + 1) * 128, :])
                    for m in range(mts):
                        msz = min(128, cap - m * 128)
                        for n in range(NT):
                            nc.tensor.matmul(
                                psums[m, n][:msz],
                                xt_sb[:, k * cap + m * 128: k * cap + m * 128 + msz],
                                w_sb[:, n * NW:(n + 1) * NW],
                                start=(k == 0),
                                stop=(k == KC - 1),
                            )
                for m in range(mts):
                    msz = min(128, cap - m * 128)
                    o_sb = o_pool.tile([128, DOUT], bf16, tag="o",
                                       name=f"o_{s}_{m}")
                    for n in range(NT):
                        nc.vector.tensor_copy(
                            o_sb[:msz, n * NW:(n + 1) * NW], psums[m, n][:msz]
                        )
                    nc.scalar.dma_start(
                        out[off + m * 128: off + m * 128 + msz, :], o_sb[:msz]
                    )
    nc.compile()
    return nc


def _run(inputs, trace=False):
    x = np.asarray(inputs["input"], dtype=np.float32)
    w = np.ascontiguousarray(np.asarray(inputs["weight"], dtype=np.float32))
    counts = np.asarray(inputs["tokens_per_expert"], dtype=np.int64)
    starts = np.concatenate([[0], np.cumsum(counts)[:-1]])

    order = np.argsort(-counts, kind="stable")  # experts by size rank
    # slot s, core c -> expert order[s*NCORES + c]; capacity = rank-group max
    caps = tuple(
        int(np.ceil(max(1, counts[order[s * NCORES:(s + 1) * NCORES]].max()) / 4) * 4)
        for s in range(EPC)
    )
    offs = np.concatenate([[0], np.cumsum(caps)]).astype(int)
    sumcap = int(offs[-1])

    if caps not in _cache:
        _cache[caps] = _build(caps)
    nc = _cache[caps]

    xs = (x * (1.0 / WSCALE)).astype(ml_dtypes.bfloat16)
    w8 = (w * WSCALE).astype(ml_dtypes.float8_e3m4)
    in_maps = []
    for c in range(NCORES):
        xt_pack = np.zeros((DIN, sumcap), dtype=ml_dtypes.bfloat16)
        w_pack = np.empty((EPC, DIN, DOUT), dtype=ml_dtypes.float8_e3m4)
        for s in range(EPC):
            g = int(order[s * NCORES + c])
            cnt = int(counts[g])
            if cnt:
                xt_pack[:, offs[s]:offs[s] + cnt] = xs[starts[g]:starts[g] + cnt].T
            w_pack[s] = w8[g]
        in_maps.append({"xt": xt_pack, "w": w_pack})

    kw = {"trace_cores": list(range(NCORES))} if trace else {}
    res = run_bass_kernel_spmd(nc, in_maps, core_ids=list(range(NCORES)),
                               trace=trace, **kw)

    out = np.empty((T, DOUT), dtype=np.float32)
    for c in range(NCORES):
        for s in range(EPC):
            g = int(order[s * NCORES + c])
            cnt = int(counts[g])
            if cnt:
                out[starts[g]:starts[g] + cnt] = \
                    res.results[c]["out"][offs[s]:offs[s] + cnt].astype(np.float32)
    return out, res


def kernel(**inputs) -> np.ndarray:
    return _run(inputs)[0]
